# revision 6
# baseline (speedup 1.0000x reference)
"""Trainium2 Bass kernel for sparse conv-transpose (gather-GEMM-scatter) + BatchNorm.

Strategy (8 NeuronCores, SPMD):
  - Shard output rows across cores (50000 rows/core). Host groups the
    2.7M (k, m) pairs by (core, k, occurrence-rank) so every scatter call
    has unique destination rows (occurrence layering makes CCE-add RMW safe).
  - Per core: indirect-DMA gather of feats rows -> PE transpose (via
    identity matmul) -> 128x128 GEMM with W[k] -> indirect-DMA scatter
    with compute_op=add into the core's raw output slice in DRAM.
  - BN stats (sum/sumsq per channel) computed on-device at the end of
    launch 1; host combines the 8 partial stats; launch 2 applies
    y = raw*scale + shift on-device.
"""

import sys

import numpy as np

sys.path.insert(0, "/opt/trn_rl_repo")

import os
import time

import concourse.bacc as bacc
import concourse.tile as tile
from concourse import bass, mybir
from concourse.bass import IndirectOffsetOnAxis
from concourse.bass_utils import run_bass_kernel_spmd

P = 128
N_CORES = 8
LAST_EXEC_NS = []  # exec_time_ns per launch (when NTFF tracing is available)
LAST_WALL_S = []   # wall seconds per launch (incl. PJRT transfer)
BLK = 4096          # pairs per staging block
CPB = BLK // P      # chunks per block (32)
def _garb(SH):
    # pad garbage region so SH+GARB is a multiple of P
    return ((-SH) % P) + P
EPS = 1e-5

f32 = mybir.dt.float32
i32 = mybir.dt.int32


def _host_prep(in_maps, out_maps, n_in, n_out):
    """Build per-core gather/scatter index streams, uniform across cores."""
    K3, M = in_maps.shape
    SH = n_out // N_CORES
    GARB = _garb(SH)
    k_all = np.repeat(np.arange(K3, dtype=np.int64), M)
    im = np.asarray(in_maps, dtype=np.int64).reshape(-1)
    om = np.asarray(out_maps, dtype=np.int64).reshape(-1)
    core = om // SH
    oml = om - core * SH

    # occurrence rank within (core, k, local-row): pairs sharing a dest row
    # within one k go to different layers -> unique dests per scatter call
    order = np.lexsort((oml, k_all, core))
    sc, sk, so, si = core[order], k_all[order], oml[order], im[order]
    new_grp = np.r_[True, (sc[1:] != sc[:-1]) | (sk[1:] != sk[:-1]) | (so[1:] != so[:-1])]
    starts = np.flatnonzero(new_grp)
    gid = np.cumsum(new_grp) - 1
    occ = np.arange(len(order)) - starts[gid]
    occ_max = int(occ.max()) + 1

    counts = np.zeros((N_CORES, K3, occ_max), np.int64)
    np.add.at(counts, (sc, sk, occ), 1)
    padded = ((counts.max(axis=0) + P - 1) // P) * P     # [K3, occ_max], shared

    # reorder pairs to (core, k, occ) grouping
    order2 = np.lexsort((occ, sk, sc))
    c2, k2, o2, i2 = sc[order2], sk[order2], so[order2], si[order2]

    gidx, sidx = [], []
    chunk_k = None
    group_bounds = None
    core_starts = np.searchsorted(c2, np.arange(N_CORES + 1))
    for c in range(N_CORES):
        s0, s1 = core_starts[c], core_starts[c + 1]
        cc_im, cc_om = i2[s0:s1], o2[s0:s1]
        cc_cnt = counts[c]
        gl, sl, ckl, cb = [], [], [], [0]
        pos = 0
        garb = 0
        for kk in range(K3):
            for rr in range(occ_max):
                n = int(cc_cnt[kk, rr])
                pn = int(padded[kk, rr])
                if pn == 0:
                    continue
                npad = pn - n
                gl.append(cc_im[pos:pos + n])
                sl.append(cc_om[pos:pos + n])
                pos += n
                if npad:
                    gl.append(np.full(npad, n_in, np.int64))
                    sl.append(SH + (np.arange(garb, garb + npad) % GARB))
                    garb += npad
                if c == 0:
                    ckl.append(np.full(pn // P, kk, np.int64))
                    cb.append(cb[-1] + pn // P)
        gidx.append(np.concatenate(gl))
        sidx.append(np.concatenate(sl))
        if c == 0:
            chunk_k = np.concatenate(ckl)
            group_bounds = cb

    TOT = len(gidx[0])
    NB = (TOT + BLK - 1) // BLK
    fill = NB * BLK - TOT
    if fill:
        for c in range(N_CORES):
            gidx[c] = np.concatenate([gidx[c], np.full(fill, n_in, np.int64)])
            sidx[c] = np.concatenate([sidx[c], SH + (np.arange(fill) % GARB)])
        chunk_k = np.concatenate([chunk_k, np.zeros(fill // P, np.int64)])
        group_bounds = group_bounds + [group_bounds[-1] + fill // P]

    nchunks = NB * CPB
    bounds = set(group_bounds)
    calls = []
    cur = 0
    for ch in range(1, nchunks + 1):
        if ch in bounds or ch % CPB == 0:
            calls.append((cur, ch))
            cur = ch

    def to_sb(a):
        # pair t=b*BLK+c*128+p lives at sbuf[p, b*CPB+c]
        return np.ascontiguousarray(
            a.astype(np.int32).reshape(NB * CPB, P).T)

    gidx = np.stack([to_sb(g) for g in gidx])
    sidx = np.stack([to_sb(s) for s in sidx])
    return dict(SH=SH, K3=K3, NB=NB, chunk_k=chunk_k, calls=calls,
                gidx=gidx, sidx=sidx)


def _build_launch1(n_in1, SH, K3, NB, chunk_k, calls):
    nc = bacc.Bacc("TRN2", target_bir_lowering=False, debug=False,
                   num_devices=N_CORES)
    feats = nc.dram_tensor("feats", [n_in1, P], f32, kind="ExternalInput")
    wcat = nc.dram_tensor("wcat", [P, K3 * P], f32, kind="ExternalInput")
    ident = nc.dram_tensor("ident", [P, P], f32, kind="ExternalInput")
    gidx_d = nc.dram_tensor("gidx", [P, NB * CPB], i32, kind="ExternalInput")
    sidx_d = nc.dram_tensor("sidx", [P, NB * CPB], i32, kind="ExternalInput")
    GARB = _garb(SH)
    raw = nc.dram_tensor("raw", [SH + GARB, P], f32, kind="ExternalOutput")
    stats = nc.dram_tensor("stats", [1, 2 * P], f32, kind="ExternalOutput")

    n_rows = SH + GARB
    assert n_rows % P == 0
    ntiles = n_rows // P
    # stats slabs: split ntiles into <=16 roughly even pieces (SBUF budget)
    nslab = min(16, ntiles)
    slab_sizes = [ntiles // nslab + (1 if i < ntiles % nslab else 0)
                  for i in range(nslab)]

    with tile.TileContext(nc) as tc:
        with tc.tile_pool(name="cst", bufs=1) as cst, \
             tc.tile_pool(name="gpool", bufs=2) as gpool, \
             tc.tile_pool(name="cpool", bufs=2) as cpool, \
             tc.tile_pool(name="gtpool", bufs=2) as gtpool, \
             tc.tile_pool(name="stat", bufs=2) as stat, \
             tc.tile_pool(name="ps", bufs=2, space="PSUM") as ps, \
             tc.tile_pool(name="ps2", bufs=2, space="PSUM") as ps2:
            w_sb = cst.tile([P, K3 * P], f32)
            nc.sync.dma_start(w_sb[:], wcat[:])
            id_sb = cst.tile([P, P], f32)
            nc.sync.dma_start(id_sb[:], ident[:])
            gidx_sb = cst.tile([P, NB * CPB], i32)
            nc.sync.dma_start(gidx_sb[:], gidx_d[:])
            sidx_sb = cst.tile([P, NB * CPB], i32)
            nc.sync.dma_start(sidx_sb[:], sidx_d[:])

            ci = 0
            for b in range(NB):
                g_st = gpool.tile([P, CPB, P], f32, tag="gst")
                for j in range(CPB):
                    col = b * CPB + j
                    nc.gpsimd.indirect_dma_start(
                        out=g_st[:, j, :], out_offset=None, in_=feats[:],
                        in_offset=IndirectOffsetOnAxis(
                            ap=gidx_sb[:, col:col + 1], axis=0))
                c_st = cpool.tile([P, CPB, P], f32, tag="cstg")
                for q in range(CPB // 4):
                    gt_ps = ps.tile([P, 4 * P], f32, tag="gtps")
                    for j4 in range(4):
                        j = q * 4 + j4
                        nc.tensor.transpose(gt_ps[:, j4 * P:(j4 + 1) * P],
                                            g_st[:, j, :], id_sb[:])
                    gt_sb = gtpool.tile([P, 4 * P], f32, tag="gtsb")
                    nc.vector.tensor_copy(gt_sb[:], gt_ps[:])
                    c_ps = ps2.tile([P, 4 * P], f32, tag="cps")
                    for j4 in range(4):
                        kk = int(chunk_k[b * CPB + q * 4 + j4])
                        nc.tensor.matmul(c_ps[:, j4 * P:(j4 + 1) * P],
                                         lhsT=gt_sb[:, j4 * P:(j4 + 1) * P],
                                         rhs=w_sb[:, kk * P:(kk + 1) * P],
                                         start=True, stop=True)
                    nc.vector.tensor_copy(c_st[:, q * 4:(q + 1) * 4, :], c_ps[:])
                for j in range(CPB):
                    col = b * CPB + j
                    nc.gpsimd.indirect_dma_start(
                        out=raw[:],
                        out_offset=IndirectOffsetOnAxis(
                            ap=sidx_sb[:, col:col + 1], axis=0),
                        in_=c_st[:, j, :],
                        in_offset=None,
                        compute_op=mybir.AluOpType.add)

            # ---- BN partial stats: sum and sum-of-squares per channel ----
            psum_t = cst.tile([P, P], f32)
            psq_t = cst.tile([P, P], f32)
            nc.gpsimd.memset(psum_t[:], 0.0)
            nc.gpsimd.memset(psq_t[:], 0.0)
            r0 = 0
            for T in slab_sizes:
                sl = stat.tile([P, T, P], f32, tag="slab")
                nc.sync.dma_start(
                    sl[:], raw[r0 * P:(r0 + T) * P, :].rearrange(
                        "(t p) c -> p t c", p=P))
                sq = stat.tile([P, T, P], f32, tag="sq")
                nc.vector.tensor_tensor(out=sq[:], in0=sl[:], in1=sl[:],
                                        op=mybir.AluOpType.mult)
                red = stat.tile([P, P], f32, tag="red")
                nc.vector.tensor_reduce(out=red[:], in_=sl[:].rearrange("p t c -> p c t"),
                                        axis=mybir.AxisListType.X,
                                        op=mybir.AluOpType.add)
                nc.vector.tensor_tensor(out=psum_t[:], in0=psum_t[:], in1=red[:],
                                        op=mybir.AluOpType.add)
                red2 = stat.tile([P, P], f32, tag="red2")
                nc.vector.tensor_reduce(out=red2[:], in_=sq[:].rearrange("p t c -> p c t"),
                                        axis=mybir.AxisListType.X,
                                        op=mybir.AluOpType.add)
                nc.vector.tensor_tensor(out=psq_t[:], in0=psq_t[:], in1=red2[:],
                                        op=mybir.AluOpType.add)
                r0 += T
            both = cst.tile([P, 2 * P], f32)
            nc.vector.tensor_copy(both[:, :P], psum_t[:])
            nc.vector.tensor_copy(both[:, P:], psq_t[:])
            ones = cst.tile([P, 1], f32)
            nc.gpsimd.memset(ones[:], 1.0)
            st_ps = ps.tile([1, 2 * P], f32, tag="stps")
            nc.tensor.matmul(st_ps[:], lhsT=ones[:], rhs=both[:],
                             start=True, stop=True)
            st_sb = cst.tile([1, 2 * P], f32)
            nc.vector.tensor_copy(st_sb[:], st_ps[:])
            nc.sync.dma_start(stats[:], st_sb[:])
    nc.compile()
    return nc


def _build_launch2(SH):
    nc = bacc.Bacc("TRN2", target_bir_lowering=False, debug=False,
                   num_devices=N_CORES)
    GARB = _garb(SH)
    raw = nc.dram_tensor("raw", [SH + GARB, P], f32, kind="ExternalInput")
    scale = nc.dram_tensor("scale", [1, P], f32, kind="ExternalInput")
    shift = nc.dram_tensor("shift", [1, P], f32, kind="ExternalInput")
    y = nc.dram_tensor("y", [SH, P], f32, kind="ExternalOutput")

    full_tiles = SH // P
    tail = SH - full_tiles * P
    nslab = min(4, max(1, full_tiles))
    slab_sizes = [full_tiles // nslab + (1 if i < full_tiles % nslab else 0)
                  for i in range(nslab)]
    with tile.TileContext(nc) as tc:
        with tc.tile_pool(name="cst", bufs=1) as cst, \
             tc.tile_pool(name="sl", bufs=2) as slp:
            sc_sb = cst.tile([P, P], f32)
            nc.sync.dma_start(sc_sb[:], scale[:].to_broadcast([P, P]))
            sh_sb = cst.tile([P, P], f32)
            nc.sync.dma_start(sh_sb[:], shift[:].to_broadcast([P, P]))
            r0 = 0
            for T in slab_sizes:
                if T == 0:
                    continue
                sl = slp.tile([P, T, P], f32, tag="slab")
                nc.sync.dma_start(
                    sl[:], raw[r0 * P:(r0 + T) * P, :].rearrange(
                        "(t p) c -> p t c", p=P))
                nc.vector.tensor_tensor(
                    out=sl[:], in0=sl[:],
                    in1=sc_sb[:, None, :].to_broadcast([P, T, P]),
                    op=mybir.AluOpType.mult)
                nc.vector.tensor_tensor(
                    out=sl[:], in0=sl[:],
                    in1=sh_sb[:, None, :].to_broadcast([P, T, P]),
                    op=mybir.AluOpType.add)
                nc.sync.dma_start(
                    y[r0 * P:(r0 + T) * P, :].rearrange("(t p) c -> p t c", p=P),
                    sl[:])
                r0 += T
            if tail:
                tl = slp.tile([P, P], f32, tag="tail")
                nc.sync.dma_start(tl[:tail, :], raw[full_tiles * P:SH, :])
                nc.vector.tensor_tensor(out=tl[:tail, :], in0=tl[:tail, :],
                                        in1=sc_sb[:tail, :],
                                        op=mybir.AluOpType.mult)
                nc.vector.tensor_tensor(out=tl[:tail, :], in0=tl[:tail, :],
                                        in1=sh_sb[:tail, :],
                                        op=mybir.AluOpType.add)
                nc.sync.dma_start(y[full_tiles * P:SH, :], tl[:tail, :])
    nc.compile()
    return nc


def kernel(feats, W, gamma, beta, in_maps, out_maps, n_out):
    feats = np.asarray(feats, np.float32)
    W = np.asarray(W, np.float32)
    gamma = np.asarray(gamma, np.float32)
    beta = np.asarray(beta, np.float32)
    in_maps = np.asarray(in_maps)
    out_maps = np.asarray(out_maps)
    n_out = int(n_out)
    n_in, C = feats.shape
    assert C == P
    K3 = W.shape[0]

    prep = _host_prep(in_maps, out_maps, n_in, n_out)
    SH, NB = prep["SH"], prep["NB"]

    feats_z = np.concatenate([feats, np.zeros((1, P), np.float32)], axis=0)
    wcat = np.ascontiguousarray(W.transpose(1, 0, 2).reshape(P, K3 * P))
    ident = np.eye(P, dtype=np.float32)

    nc1 = _build_launch1(n_in + 1, SH, K3, NB, prep["chunk_k"], prep["calls"])
    in_maps1 = [dict(feats=feats_z, wcat=wcat, ident=ident,
                     gidx=np.ascontiguousarray(prep["gidx"][c]),
                     sidx=np.ascontiguousarray(prep["sidx"][c]))
                for c in range(N_CORES)]
    _trace = os.environ.get("BASS_KERNEL_TRACE") == "1"
    LAST_EXEC_NS.clear()
    LAST_WALL_S.clear()
    _t = time.time()
    try:
        res1 = run_bass_kernel_spmd(nc1, in_maps1,
                                    core_ids=list(range(N_CORES)),
                                    trace=_trace)
    except ModuleNotFoundError:
        res1 = run_bass_kernel_spmd(nc1, in_maps1,
                                    core_ids=list(range(N_CORES)))
    LAST_WALL_S.append(time.time() - _t)
    if res1.exec_time_ns is not None:
        LAST_EXEC_NS.append(res1.exec_time_ns)
    raws = [res1.results[c]["raw"] for c in range(N_CORES)]
    stats = np.stack([res1.results[c]["stats"].reshape(2, P)
                      for c in range(N_CORES)])

    tot_sum = stats[:, 0, :].sum(axis=0)
    tot_sq = stats[:, 1, :].sum(axis=0)
    mean = tot_sum / n_out
    var = tot_sq / n_out - mean * mean
    scale = (gamma / np.sqrt(var + EPS)).astype(np.float32)
    shift = (beta - mean * scale).astype(np.float32)

    nc2 = _build_launch2(SH)
    in_maps2 = [dict(raw=raws[c], scale=scale.reshape(1, P),
                     shift=shift.reshape(1, P)) for c in range(N_CORES)]
    _t = time.time()
    try:
        res2 = run_bass_kernel_spmd(nc2, in_maps2,
                                    core_ids=list(range(N_CORES)),
                                    trace=_trace)
    except ModuleNotFoundError:
        res2 = run_bass_kernel_spmd(nc2, in_maps2,
                                    core_ids=list(range(N_CORES)))
    LAST_WALL_S.append(time.time() - _t)
    if res2.exec_time_ns is not None:
        LAST_EXEC_NS.append(res2.exec_time_ns)
    y = np.concatenate([res2.results[c]["y"] for c in range(N_CORES)], axis=0)
    return y
